# revision 9
# baseline (speedup 1.0000x reference)
"""Trainium2 Bass kernel for a GPT-OSS-style MoE MLP block (top-2 of 8 experts).

Strategy (expert-parallel with chunk-level load balancing, full_io):
  - Host computes router softmax + top-2 + renormalized combine weights.
  - Each expert is owned by one core.  Experts with more than LIGHT_W=512
    tokens ("heavy") donate a few intermediate-dim chunks (full token width)
    to the light cores; each light core receives at most one 288-column
    half-chunk.  Partial down-projection outputs simply add up on the host.
  - Three body CLASSES via If(partition_id) — light (6 cores, identical
    instructions, per-core data), and one class per heavy core.  Untaken
    If bodies are walked by the sequencers at ~2ns/instr, so few classes
    matter; all matmuls are >= 288 wide so no ldweights exposure.
  - Per unit: gate/up matmuls -> h = combine_w * SiLU(gate) * up (bf16 in,
    fp32 accumulate); then down matmul -> weighted partial outputs yT.
  - The last (half-size) gate and up output chunks are packed into a single
    matmul (64+64 partitions), saving one output chunk-row of work.
  - Dummy warm-up matmuls run during the initial token/weight DMA so the PE
    p-state ramp completes before real work arrives.

Device-side layouts (per core):
  tT  : [23, 128, CT]      bf16  tokens^T: own tokens in cols [0:OWN_W),
                                 foreign half-chunk tokens in [OWN_W:CT)
  wg  : [23, 128, 23, 128] bf16  [i_blk][d_part][d_chunk][i_in_blk]; the
                                 i_blk=22 entry packs gate[2816:2880] in
                                 cols 0:64 and up[2816:2880] in cols 64:128
  wu  : [22, 128, 23, 128] bf16  up chunks 0..21
  wd  : [23, 128, 23, 128] bf16  [d_chunk][i_part][i_chunk][d_in_chunk]
  wfg/wfu : [128, 23, 128] bf16  foreign chunk gate/up weights
  wfd : [128, 23, 128]     bf16  [i_in_chunk][d_chunk][d_in_chunk] foreign
                                 chunk down rows
  wvr : [128, CT]          f32   combine weights replicated per partition
  yT  : [23, 128, CT]      f32   yT[dc,dp,c] = y[c, dc*128+dp]
"""

import os

import ml_dtypes
import numpy as np

T, D, E, TOPK = 2048, 2880, 8, 2
P = 128
DP = 2944  # D and I padded to 23*128
KD = DP // P  # 23 contraction chunks for gate/up
KI = DP // P  # 23 intermediate chunks
KO = DP // P  # 23 output-D chunks (padded)
N_CORES = 8
IHALF = 64   # valid rows in the last (packed) I chunk: 2880 - 22*128
LIGHT_W = 512
FOR_W = 288  # foreign half-chunk width

BF16 = ml_dtypes.bfloat16

_cache = {}


def _route(x, w_router):
    """Host top-2 routing, mirroring the jax reference numerics."""
    t = np.ascontiguousarray(x.reshape(-1, D).astype(np.float32))
    logits = t @ w_router.astype(np.float32)  # [T, E]
    m = logits.max(axis=-1, keepdims=True)
    ex = np.exp(logits - m)
    aff = ex / ex.sum(axis=-1, keepdims=True)
    i1 = aff.argmax(axis=-1)
    a2 = aff.copy()
    a2[np.arange(aff.shape[0]), i1] = -np.inf
    i2 = a2.argmax(axis=-1)
    v1 = aff[np.arange(aff.shape[0]), i1]
    v2 = aff[np.arange(aff.shape[0]), i2]
    s = v1 + v2
    return t, i1, i2, v1 / s, v2 / s


def _blocks(total, max_bs=512):
    """Near-equal column blocks of width <= max_bs (each >= 128 when
    total >= 128, so matmul weight loads stay hidden)."""
    nb = (total + max_bs - 1) // max_bs
    bs = (total + nb - 1) // nb
    out = []
    off = 0
    while off < total:
        w = min(bs, total - off)
        out.append((off, w))
        off += w
    return out


def _schedule(counts):
    """Assign each expert to its core; heavy experts (> LIGHT_W tokens)
    donate whole I-chunks, split into <=2 FOR_W-wide half-chunks, one per
    light core.  Minimizes the max per-core row-load.

    Returns (classes, donations) where classes[k] describes core k's body
    class and donations[k] is the foreign half-chunk for light core k."""
    heavy = [e for e in range(E) if counts[e] > LIGHT_W]
    light = [e for e in range(E) if counts[e] <= LIGHT_W]
    base_light = 68 * LIGHT_W  # row-units of a light body without foreign
    half_rows = 3 * FOR_W

    # choose donation counts z_e minimizing the max load
    best = None
    zmax = [min(7, (2 * len(light)) // max(1, 2 * len(heavy)) + 3)
            for _ in heavy]
    import itertools
    for zs in itertools.product(*[range(0, zm + 1) for zm in zmax]):
        halves = sum(2 * z for z in zs)
        if halves > len(light):
            continue
        loads = [(68 - 3 * z) * counts[e] for z, e in zip(zs, heavy)]
        loads.append(base_light + (half_rows if halves > 0 else 0))
        cand = (max(loads), halves)
        if best is None or cand < best[0]:
            best = (cand, zs)
    zs = best[1] if best else [0] * len(heavy)

    donations = []  # (expert, chunk_ib, col0, ncols) per light slot
    for e, z in zip(heavy, zs):
        n = counts[e]
        for ib in range(z):
            donations.append((e, ib, 0, min(FOR_W, n)))
            if n > FOR_W:
                donations.append((e, ib, FOR_W, min(FOR_W, n - FOR_W)))
    assert len(donations) <= len(light)
    classes = {}
    for i, e in enumerate(light):
        classes[e] = {"kind": "light", "expert": e,
                      "foreign": donations[i] if i < len(donations) else None}
    for e, z in zip(heavy, zs):
        classes[e] = {"kind": "heavy", "expert": e, "donated": z,
                      "width": counts[e]}
    # core k runs expert k's body; lights listed first in pid order
    order = light + heavy
    return classes, order, zs, heavy, light


def _build_program(spec_key, specs, ctmax, h_w):
    import concourse.bacc as bacc
    import concourse.mybir as mybir
    import concourse.tile as tile

    f32 = mybir.dt.float32
    bf16 = mybir.dt.bfloat16
    SILU = mybir.ActivationFunctionType.Silu

    nc = bacc.Bacc("TRN2", target_bir_lowering=False, debug=False,
                   num_devices=N_CORES)

    tT_d = nc.dram_tensor("tT", [KD, P, ctmax], bf16,
                          kind="ExternalInput").ap()
    wg_d = nc.dram_tensor("wg", [KI, P, KD, P], bf16,
                          kind="ExternalInput").ap()
    wu_d = nc.dram_tensor("wu", [KI - 1, P, KD, P], bf16,
                          kind="ExternalInput").ap()
    wd_d = nc.dram_tensor("wd", [KO, P, KI, P], bf16,
                          kind="ExternalInput").ap()
    wfg_d = nc.dram_tensor("wfg", [P, KD, P], bf16,
                           kind="ExternalInput").ap()
    wfu_d = nc.dram_tensor("wfu", [P, KD, P], bf16,
                           kind="ExternalInput").ap()
    wfd_d = nc.dram_tensor("wfd", [P, KO, P], bf16,
                           kind="ExternalInput").ap()
    wvr_d = nc.dram_tensor("wvr", [P, ctmax], f32, kind="ExternalInput").ap()
    yT_d = nc.dram_tensor("yT", [KO, P, ctmax], f32,
                          kind="ExternalOutput").ap()

    with tile.TileContext(nc) as tc:
        with tc.tile_pool(name="resident", bufs=1) as res_pool, \
             tc.tile_pool(name="wgu", bufs=3) as wgu_pool, \
             tc.tile_pool(name="wdp", bufs=3) as wd_pool, \
             tc.tile_pool(name="tmp", bufs=2) as tmp_pool, \
             tc.tile_pool(name="yev", bufs=3) as y_pool, \
             tc.tile_pool(name="ps", bufs=2, space="PSUM") as ps_pool:

            h = [res_pool.tile([P, h_w], bf16, tag=f"h{ib}",
                               name=f"h_{ib}") for ib in range(KI)]
            hF = res_pool.tile([P, FOR_W], bf16, tag="hF")
            tok = [res_pool.tile([P, ctmax], bf16, tag=f"tok{dk}",
                                 name=f"tok_{dk}") for dk in range(KD)]
            wvr = res_pool.tile([P, ctmax], f32, tag="wvr")
            wfd_sb = res_pool.tile([P, KO, P], bf16, tag="wfd")
            warm = res_pool.tile([P, 192], bf16, tag="warm")

            # PE p-state warm-up while the first token/weight DMAs stream.
            nc.gpsimd.memset(warm, 0.0)
            for wi in range(40):
                ps_w = ps_pool.tile([P, 512], f32, tag="pg0",
                                    name=f"warm_{wi}")
                nc.tensor.matmul(ps_w[:, :192], lhsT=warm[:, :P], rhs=warm,
                                 start=True, stop=True)
            # packed-chunk upper partitions are read (x0 weights) by the
            # down matmul: must not be NaN garbage
            nc.vector.memset(h[KI - 1][IHALF:, :], 0.0)

            pid = nc.partition_id()

            def emit_body(cls, width, has_foreign, skip):
                """width = own token width; skip = donated leading chunks
                (heavy classes) excluded from phase 1 and the down
                contraction."""
                cblk = _blocks(width)
                nblk = len(cblk)
                tok_w = width + (FOR_W if has_foreign else 0)

                # ---- phase 1 ----
                wd_pre = []
                for ib in range(skip, KI):
                    packed = ib == KI - 1
                    wg_blk = wgu_pool.tile([P, KD, P], bf16, tag="wg",
                                           name=f"wg_{cls}_{ib}")
                    nc.sync.dma_start(out=wg_blk, in_=wg_d[ib])
                    if ib == skip:
                        for dk in range(KD):
                            nc.sync.dma_start(out=tok[dk][:, :width],
                                              in_=tT_d[dk][:, :width])
                        nc.sync.dma_start(out=wvr[:, :tok_w],
                                          in_=wvr_d[:, :tok_w])
                    if has_foreign and ib == KI - 4:
                        # foreign-unit data, needed only after the own loop
                        for dk in range(KD):
                            nc.sync.dma_start(
                                out=tok[dk][:, width:tok_w],
                                in_=tT_d[dk][:, width:tok_w])
                        nc.sync.dma_start(out=wfd_sb, in_=wfd_d)
                    wu_blk = None
                    if not packed:
                        wu_blk = wgu_pool.tile([P, KD, P], bf16, tag="wu",
                                               name=f"wu_{cls}_{ib}")
                        nc.sync.dma_start(out=wu_blk, in_=wu_d[ib])
                    if ib == KI - 3:
                        wd_blk = wd_pool.tile([P, KI, P], bf16, tag="wd",
                                              name=f"wd_{cls}_pre")
                        nc.sync.dma_start(out=wd_blk, in_=wd_d[0])
                        wd_pre.append(wd_blk)

                    ps_g = [ps_pool.tile([P, 512], f32, tag=f"pg{bi}",
                                         name=f"psg{bi}_{cls}_{ib}")
                            for bi in range(nblk)]
                    ps_u = [ps_pool.tile([P, 512], f32, tag=f"pu{bi}",
                                         name=f"psu{bi}_{cls}_{ib}")
                            for bi in range(nblk)] if not packed else []
                    for bi, (b0, bw) in enumerate(cblk):
                        for dk in range(KD):
                            nc.tensor.matmul(
                                ps_g[bi][:, :bw], lhsT=wg_blk[:, dk, :],
                                rhs=tok[dk][:, b0:b0 + bw],
                                start=dk == 0, stop=dk == KD - 1)
                    if not packed:
                        for bi, (b0, bw) in enumerate(cblk):
                            for dk in range(KD):
                                nc.tensor.matmul(
                                    ps_u[bi][:, :bw], lhsT=wu_blk[:, dk, :],
                                    rhs=tok[dk][:, b0:b0 + bw],
                                    start=dk == 0, stop=dk == KD - 1)
                    for bi, (b0, bw) in enumerate(cblk):
                        if packed:
                            tmp = tmp_pool.tile([IHALF, 512], f32,
                                                tag=f"t{bi}",
                                                name=f"tp{bi}_{cls}_{ib}")
                            nc.scalar.activation(tmp[:, :bw],
                                                 ps_g[bi][:IHALF, :bw], SILU)
                            tmp2 = tmp_pool.tile([IHALF, 512], f32,
                                                 tag=f"t2{bi}",
                                                 name=f"tq{bi}_{cls}_{ib}")
                            nc.vector.tensor_mul(tmp2[:, :bw], tmp[:, :bw],
                                                 ps_g[bi][IHALF:, :bw])
                            nc.vector.tensor_mul(
                                h[ib][:IHALF, b0:b0 + bw], tmp2[:, :bw],
                                wvr[:IHALF, b0:b0 + bw])
                        else:
                            tmp = tmp_pool.tile([P, 512], f32, tag=f"t{bi}",
                                                name=f"tp{bi}_{cls}_{ib}")
                            nc.scalar.activation(tmp[:, :bw],
                                                 ps_g[bi][:, :bw], SILU)
                            tmp2 = tmp_pool.tile([P, 512], f32, tag=f"t2{bi}",
                                                 name=f"tq{bi}_{cls}_{ib}")
                            nc.vector.tensor_mul(tmp2[:, :bw], tmp[:, :bw],
                                                 ps_u[bi][:, :bw])
                            nc.vector.tensor_mul(
                                h[ib][:, b0:b0 + bw], tmp2[:, :bw],
                                wvr[:, b0:b0 + bw])

                # foreign half-chunk: gate/up over cols [own_w:own_w+FOR_W)
                if has_foreign:
                    wfg_t = wgu_pool.tile([P, KD, P], bf16, tag="wg",
                                          name=f"wfg_{cls}")
                    nc.sync.dma_start(out=wfg_t, in_=wfg_d)
                    wfu_t = wgu_pool.tile([P, KD, P], bf16, tag="wu",
                                          name=f"wfu_{cls}")
                    nc.sync.dma_start(out=wfu_t, in_=wfu_d)
                    ps_fg = ps_pool.tile([P, 512], f32, tag="pg0",
                                         name=f"psfg_{cls}")
                    ps_fu = ps_pool.tile([P, 512], f32, tag="pu0",
                                         name=f"psfu_{cls}")
                    for dk in range(KD):
                        nc.tensor.matmul(
                            ps_fg[:, :FOR_W], lhsT=wfg_t[:, dk, :],
                            rhs=tok[dk][:, width:width + FOR_W],
                            start=dk == 0, stop=dk == KD - 1)
                    for dk in range(KD):
                        nc.tensor.matmul(
                            ps_fu[:, :FOR_W], lhsT=wfu_t[:, dk, :],
                            rhs=tok[dk][:, width:width + FOR_W],
                            start=dk == 0, stop=dk == KD - 1)
                    tmp = tmp_pool.tile([P, 512], f32, tag="t0",
                                        name=f"tpf_{cls}")
                    nc.scalar.activation(tmp[:, :FOR_W], ps_fg[:, :FOR_W],
                                         SILU)
                    tmp2 = tmp_pool.tile([P, 512], f32, tag="t20",
                                         name=f"tqf_{cls}")
                    nc.vector.tensor_mul(tmp2[:, :FOR_W], tmp[:, :FOR_W],
                                         ps_fu[:, :FOR_W])
                    nc.vector.tensor_mul(hF, tmp2[:, :FOR_W],
                                         wvr[:, width:width + FOR_W])

                # ---- phase 2: down matmul -> yT ----
                for dc in range(KO):
                    if dc == 0:
                        wd_blk = wd_pre[0]
                    else:
                        wd_blk = wd_pool.tile([P, KI, P], bf16, tag="wd",
                                              name=f"wd_{cls}_{dc}")
                        nc.sync.dma_start(out=wd_blk, in_=wd_d[dc])
                    ps_y = [ps_pool.tile([P, 512], f32, tag=f"pg{bi}",
                                         name=f"psy{bi}_{cls}_{dc}")
                            for bi in range(nblk)]
                    for bi, (b0, bw) in enumerate(cblk):
                        for ib in range(skip, KI):
                            nc.tensor.matmul(
                                ps_y[bi][:, :bw], lhsT=wd_blk[:, ib, :],
                                rhs=h[ib][:, b0:b0 + bw],
                                start=ib == skip, stop=ib == KI - 1)
                    if has_foreign:
                        ps_yf = ps_pool.tile([P, 512], f32, tag="pu0",
                                             name=f"psyf_{cls}_{dc}")
                        nc.tensor.matmul(ps_yf[:, :FOR_W],
                                         lhsT=wfd_sb[:, dc, :], rhs=hF,
                                         start=True, stop=True)
                    y_sb = y_pool.tile([P, ctmax], f32, tag="ysb",
                                       name=f"ysb_{cls}_{dc}")
                    for bi, (b0, bw) in enumerate(cblk):
                        nc.scalar.copy(y_sb[:, b0:b0 + bw], ps_y[bi][:, :bw])
                    if has_foreign:
                        nc.scalar.copy(y_sb[:, width:width + FOR_W],
                                       ps_yf[:, :FOR_W])
                    nc.sync.dma_start(out=yT_d[dc][:, :tok_w],
                                      in_=y_sb[:, :tok_w])

            # one If-region per body class; cores of the same class run
            # identical instructions on per-core data.  Heavy classes are
            # emitted first and the light class last: sequencers walk
            # untaken bodies (~2ns/instr), and a walk BEFORE a core's own
            # body hides under its DMA head while a walk after is pure tail
            # -- so the class with the most cores (light) goes last and
            # pays no tail.
            emit_order = sorted(specs, key=lambda s: (s["cls"] == "light",
                                                      s["width"]))
            for ci, sp in enumerate(emit_order):
                lo, hi = sp["pid_range"]
                cond = (pid < hi) if hi - lo > 1 else (pid == lo)
                with tc.If(cond):
                    emit_body(sp["cls"], sp["width"], sp["has_foreign"],
                              sp["skip"])

    nc.compile()
    return nc


def _prep_expert(w_gate_e, w_up_e, w_down_e):
    """Per-expert device weight layouts (packed last gate/up chunk)."""
    wg = np.zeros((DP, DP), np.float32)
    wg[:D, :D] = w_gate_e
    wu = np.zeros((DP, DP), np.float32)
    wu[:D, :D] = w_up_e
    # pack: last I chunk keeps gate[2816:2880] in cols 0:64 and gains
    # up[2816:2880] in cols 64:128
    wg[:, DP - P + IHALF:] = wu[:, DP - P:DP - P + IHALF]
    wgp = np.ascontiguousarray(
        wg.reshape(KD, P, KI, P).transpose(2, 1, 0, 3)).astype(BF16)
    wup = np.ascontiguousarray(
        wu.reshape(KD, P, KI, P).transpose(2, 1, 0, 3)[:KI - 1]).astype(BF16)

    wd = np.zeros((DP, DP), np.float32)
    wd[:D, :D] = w_down_e
    wdp = np.ascontiguousarray(
        wd.reshape(KI, P, KO, P).transpose(2, 1, 0, 3)).astype(BF16)
    return wgp, wup, wdp


def moe_forward(x, w_router, w_gate, w_up, w_down, trace=False):
    from concourse.bass_utils import run_bass_kernel_spmd

    x = np.asarray(x)
    t, i1, i2, w1, w2 = _route(x, np.asarray(w_router))
    Ttok = t.shape[0]

    idx_list, wv_list = [], []
    for e in range(E):
        sel1 = i1 == e
        sel2 = i2 == e
        idx = np.nonzero(sel1 | sel2)[0]
        w = np.where(sel1[idx], w1[idx], w2[idx]).astype(np.float32)
        idx_list.append(idx)
        wv_list.append(w)
    counts = [len(ix) for ix in idx_list]

    classes, order, zs, heavy, light = _schedule(counts)
    # pid layout: lights at pids [0, len(light)), then heavies
    # body specs: one light class + one class per heavy expert
    any_foreign = any(classes[e]["foreign"] is not None for e in light)
    hw_max = max((counts[e] for e in heavy), default=0)
    ctmax = max(LIGHT_W + (FOR_W if any_foreign else 0), hw_max)
    ctmax = (ctmax + 31) // 32 * 32

    specs = []
    if light:
        specs.append({"cls": "light", "width": LIGHT_W,
                      "has_foreign": any_foreign, "skip": 0,
                      "pid_range": (0, len(light))})
    for i, e in enumerate(heavy):
        specs.append({"cls": f"heavy{e}", "width": counts[e],
                      "has_foreign": False, "skip": classes[e]["donated"],
                      "pid_range": (len(light) + i, len(light) + i + 1)})
    h_w = max(s["width"] for s in specs)
    spec_key = tuple((s["cls"], s["width"], s["has_foreign"], s["skip"])
                     for s in specs)

    key = (spec_key, ctmax)
    if key not in _cache:
        _cache[key] = _build_program(spec_key, specs, ctmax, h_w)
    nc = _cache[key]

    wg_f = np.asarray(w_gate, np.float32)
    wu_f = np.asarray(w_up, np.float32)
    wd_f = np.asarray(w_down, np.float32)
    experts = {e: _prep_expert(wg_f[e], wu_f[e], wd_f[e]) for e in range(E)}

    zdum_g = np.zeros((P, KD, P), BF16)
    zdum_d = np.zeros((P, KO, P), BF16)

    in_maps = []
    core_info = []  # (own tok ids, n_own, foreign tok ids or None)
    for k in range(N_CORES):
        e = order[k]
        cl = classes[e]
        own_ids = idx_list[e]
        own_w = LIGHT_W if cl["kind"] == "light" else counts[e]
        wvals = wv_list[e]

        n = len(own_ids)
        tpad = np.zeros((ctmax, DP), np.float32)
        tpad[:n, :D] = t[own_ids]
        wv = np.zeros((ctmax,), np.float32)
        wv[:n] = wvals

        g1, u1, d1 = experts[e]
        fm = {"wg": g1, "wu": u1, "wd": d1,
              "wfg": zdum_g, "wfu": zdum_g, "wfd": zdum_d}
        f_ids = None
        if cl["kind"] == "light" and cl["foreign"] is not None:
            fe, fib, fb0, fbw = cl["foreign"]
            f_ids = idx_list[fe][fb0:fb0 + fbw]
            nf = len(f_ids)
            tpad[own_w:own_w + nf, :D] = t[f_ids]
            wv[own_w:own_w + nf] = wv_list[fe][fb0:fb0 + fbw]
            g2, u2, d2 = experts[fe]
            fm["wfg"] = np.ascontiguousarray(g2[fib])
            fm["wfu"] = np.ascontiguousarray(u2[fib])
            fm["wfd"] = np.ascontiguousarray(
                d2[:, :, fib, :].transpose(1, 0, 2))

        tT = np.ascontiguousarray(tpad.T).reshape(KD, P, ctmax).astype(BF16)
        wvr = np.ascontiguousarray(np.broadcast_to(wv, (P, ctmax)))
        fm.update({"tT": tT, "wvr": wvr})
        in_maps.append(fm)
        core_info.append((own_ids, n, own_w, f_ids))

    try:
        res = run_bass_kernel_spmd(nc, in_maps, list(range(N_CORES)),
                                   trace=trace)
    except Exception:
        # transient NRT/device hiccups have been observed; retry once
        res = run_bass_kernel_spmd(nc, in_maps, list(range(N_CORES)),
                                   trace=trace)

    out = np.zeros((Ttok, D), np.float32)
    for k in range(N_CORES):
        own_ids, n, own_w, f_ids = core_info[k]
        yT = res.results[k]["yT"].reshape(DP, -1)  # [dc*128+dp, c]
        np.add.at(out, own_ids, yT[:D, :n].T)
        if f_ids is not None and len(f_ids):
            np.add.at(out, f_ids, yT[:D, own_w:own_w + len(f_ids)].T)

    return out.reshape(x.shape).astype(np.float32), res


def kernel(x, w_router, w_gate, w_up, w_down):
    out, _ = moe_forward(x, w_router, w_gate, w_up, w_down,
                         trace=bool(int(os.environ.get("MOE_TRACE", "0"))))
    return out


# revision 10
# speedup vs baseline: 1.0420x; 1.0420x over previous
"""Trainium2 Bass kernel for a GPT-OSS-style MoE MLP block (top-2 of 8 experts).

Strategy (expert-parallel with chunk-level load balancing, full_io):
  - Host computes router softmax + top-2 + renormalized combine weights.
  - Each expert is owned by one core.  Experts with more than LIGHT_W=512
    tokens ("heavy") donate a few intermediate-dim chunks (full token width)
    to the light cores; each light core receives at most one 288-column
    half-chunk.  Partial down-projection outputs simply add up on the host.
  - Three body CLASSES via If(partition_id) — light (6 cores, identical
    instructions, per-core data), and one class per heavy core.  Untaken
    If bodies are walked by the sequencers at ~2ns/instr, so few classes
    matter; all matmuls are >= 288 wide so no ldweights exposure.
  - Per unit: gate/up matmuls -> h = combine_w * SiLU(gate) * up (bf16 in,
    fp32 accumulate); then down matmul -> weighted partial outputs yT.
  - The last (half-size) gate and up output chunks are packed into a single
    matmul (64+64 partitions), saving one output chunk-row of work.
  - Dummy warm-up matmuls run during the initial token/weight DMA so the PE
    p-state ramp completes before real work arrives.

Device-side layouts (per core):
  tT  : [23, 128, CT]      bf16  tokens^T: own tokens in cols [0:OWN_W),
                                 foreign half-chunk tokens in [OWN_W:CT)
  wg  : [23, 128, 23, 128] bf16  [i_blk][d_part][d_chunk][i_in_blk]; the
                                 i_blk=22 entry packs gate[2816:2880] in
                                 cols 0:64 and up[2816:2880] in cols 64:128
  wu  : [22, 128, 23, 128] bf16  up chunks 0..21
  wd  : [23, 128, 23, 128] bf16  [d_chunk][i_part][i_chunk][d_in_chunk]
  wfg/wfu : [128, 23, 128] bf16  foreign chunk gate/up weights
  wfd : [128, 23, 128]     bf16  [i_in_chunk][d_chunk][d_in_chunk] foreign
                                 chunk down rows
  wvr : [128, CT]          f32   combine weights replicated per partition
  yT  : [23, 128, CT]      f32   yT[dc,dp,c] = y[c, dc*128+dp]
"""

import os

import ml_dtypes
import numpy as np

T, D, E, TOPK = 2048, 2880, 8, 2
P = 128
DP = 2944  # D and I padded to 23*128
KD = DP // P  # 23 contraction chunks for gate/up
KI = DP // P  # 23 intermediate chunks
KO = DP // P  # 23 output-D chunks (padded)
N_CORES = 8
IHALF = 64   # valid rows in the last (packed) I chunk: 2880 - 22*128
LIGHT_W = 512
FOR_W = 288  # foreign half-chunk width

BF16 = ml_dtypes.bfloat16

_cache = {}


def _route(x, w_router):
    """Host top-2 routing, mirroring the jax reference numerics."""
    t = np.ascontiguousarray(x.reshape(-1, D).astype(np.float32))
    logits = t @ w_router.astype(np.float32)  # [T, E]
    m = logits.max(axis=-1, keepdims=True)
    ex = np.exp(logits - m)
    aff = ex / ex.sum(axis=-1, keepdims=True)
    i1 = aff.argmax(axis=-1)
    a2 = aff.copy()
    a2[np.arange(aff.shape[0]), i1] = -np.inf
    i2 = a2.argmax(axis=-1)
    v1 = aff[np.arange(aff.shape[0]), i1]
    v2 = aff[np.arange(aff.shape[0]), i2]
    s = v1 + v2
    return t, i1, i2, v1 / s, v2 / s


def _blocks(total, max_bs=512):
    """Near-equal column blocks of width <= max_bs (each >= 128 when
    total >= 128, so matmul weight loads stay hidden)."""
    nb = (total + max_bs - 1) // max_bs
    bs = (total + nb - 1) // nb
    out = []
    off = 0
    while off < total:
        w = min(bs, total - off)
        out.append((off, w))
        off += w
    return out


def _schedule(counts):
    """Assign each expert to its core; heavy experts (> LIGHT_W tokens)
    donate whole I-chunks, split into <=2 FOR_W-wide half-chunks, one per
    light core.  Minimizes the max per-core row-load.

    Returns (classes, donations) where classes[k] describes core k's body
    class and donations[k] is the foreign half-chunk for light core k."""
    heavy = [e for e in range(E) if counts[e] > LIGHT_W]
    light = [e for e in range(E) if counts[e] <= LIGHT_W]
    base_light = 68 * LIGHT_W  # row-units of a light body without foreign
    half_rows = 3 * FOR_W

    # choose donation counts z_e minimizing the max load
    best = None
    zmax = [min(7, (2 * len(light)) // max(1, 2 * len(heavy)) + 3)
            for _ in heavy]
    import itertools
    for zs in itertools.product(*[range(0, zm + 1) for zm in zmax]):
        halves = sum(2 * z for z in zs)
        if halves > len(light):
            continue
        loads = [(68 - 3 * z) * counts[e] for z, e in zip(zs, heavy)]
        loads.append(base_light + (half_rows if halves > 0 else 0))
        cand = (max(loads), halves)
        if best is None or cand < best[0]:
            best = (cand, zs)
    zs = best[1] if best else [0] * len(heavy)

    donations = []  # (expert, chunk_ib, col0, ncols) per light slot
    for e, z in zip(heavy, zs):
        n = counts[e]
        for ib in range(z):
            donations.append((e, ib, 0, min(FOR_W, n)))
            if n > FOR_W:
                donations.append((e, ib, FOR_W, min(FOR_W, n - FOR_W)))
    assert len(donations) <= len(light)
    classes = {}
    for i, e in enumerate(light):
        classes[e] = {"kind": "light", "expert": e,
                      "foreign": donations[i] if i < len(donations) else None}
    for e, z in zip(heavy, zs):
        classes[e] = {"kind": "heavy", "expert": e, "donated": z,
                      "width": counts[e]}
    # core k runs expert k's body; lights listed first in pid order
    order = light + heavy
    return classes, order, zs, heavy, light


def _build_program(spec_key, specs, ctmax, h_w):
    import concourse.bacc as bacc
    import concourse.mybir as mybir
    import concourse.tile as tile

    f32 = mybir.dt.float32
    bf16 = mybir.dt.bfloat16
    SILU = mybir.ActivationFunctionType.Silu

    nc = bacc.Bacc("TRN2", target_bir_lowering=False, debug=False,
                   num_devices=N_CORES)

    tT_d = nc.dram_tensor("tT", [KD, P, ctmax], bf16,
                          kind="ExternalInput").ap()
    wg_d = nc.dram_tensor("wg", [KI, P, KD, P], bf16,
                          kind="ExternalInput").ap()
    wu_d = nc.dram_tensor("wu", [KI - 1, P, KD, P], bf16,
                          kind="ExternalInput").ap()
    wd_d = nc.dram_tensor("wd", [KO, P, KI, P], bf16,
                          kind="ExternalInput").ap()
    wfg_d = nc.dram_tensor("wfg", [P, KD, P], bf16,
                           kind="ExternalInput").ap()
    wfu_d = nc.dram_tensor("wfu", [P, KD, P], bf16,
                           kind="ExternalInput").ap()
    wfd_d = nc.dram_tensor("wfd", [P, KO, P], bf16,
                           kind="ExternalInput").ap()
    wvr_d = nc.dram_tensor("wvr", [P, ctmax], f32, kind="ExternalInput").ap()
    yT_d = nc.dram_tensor("yT", [KO, P, ctmax], f32,
                          kind="ExternalOutput").ap()

    with tile.TileContext(nc) as tc:
        with tc.tile_pool(name="resident", bufs=1) as res_pool, \
             tc.tile_pool(name="wgu", bufs=3) as wgu_pool, \
             tc.tile_pool(name="wdp", bufs=3) as wd_pool, \
             tc.tile_pool(name="tmp", bufs=2) as tmp_pool, \
             tc.tile_pool(name="yev", bufs=3) as y_pool, \
             tc.tile_pool(name="ps", bufs=2, space="PSUM") as ps_pool:

            h = [res_pool.tile([P, h_w], bf16, tag=f"h{ib}",
                               name=f"h_{ib}") for ib in range(KI)]
            hF = res_pool.tile([P, FOR_W], bf16, tag="hF")
            tok = [res_pool.tile([P, ctmax], bf16, tag=f"tok{dk}",
                                 name=f"tok_{dk}") for dk in range(KD)]
            wvr = res_pool.tile([P, ctmax], f32, tag="wvr")
            wfd_sb = res_pool.tile([P, KO, P], bf16, tag="wfd")
            warm = res_pool.tile([P, 192], bf16, tag="warm")

            # PE p-state warm-up while the first token/weight DMAs stream.
            nc.gpsimd.memset(warm, 0.0)
            for wi in range(40):
                ps_w = ps_pool.tile([P, 512], f32, tag="pg0",
                                    name=f"warm_{wi}")
                nc.tensor.matmul(ps_w[:, :192], lhsT=warm[:, :P], rhs=warm,
                                 start=True, stop=True)
            # packed-chunk upper partitions are read (x0 weights) by the
            # down matmul: must not be NaN garbage
            nc.vector.memset(h[KI - 1][IHALF:, :], 0.0)

            pid = nc.partition_id()

            def emit_body(cls, width, has_foreign, skip):
                """width = own token width; skip = donated leading chunks
                (heavy classes) excluded from phase 1 and the down
                contraction."""
                cblk = _blocks(width)
                nblk = len(cblk)
                tok_w = width + (FOR_W if has_foreign else 0)

                # ---- phase 1 ----
                wd_pre = []
                for ib in range(skip, KI):
                    packed = ib == KI - 1
                    wg_blk = wgu_pool.tile([P, KD, P], bf16, tag="wg",
                                           name=f"wg_{cls}_{ib}")
                    nc.sync.dma_start(out=wg_blk, in_=wg_d[ib])
                    if ib == skip:
                        for dk in range(KD):
                            nc.sync.dma_start(out=tok[dk][:, :width],
                                              in_=tT_d[dk][:, :width])
                        nc.sync.dma_start(out=wvr[:, :tok_w],
                                          in_=wvr_d[:, :tok_w])
                    if has_foreign and ib == KI - 4:
                        # foreign-unit data, needed only after the own loop
                        for dk in range(KD):
                            nc.sync.dma_start(
                                out=tok[dk][:, width:tok_w],
                                in_=tT_d[dk][:, width:tok_w])
                        nc.sync.dma_start(out=wfd_sb, in_=wfd_d)
                    wu_blk = None
                    if not packed:
                        wu_blk = wgu_pool.tile([P, KD, P], bf16, tag="wu",
                                               name=f"wu_{cls}_{ib}")
                        nc.sync.dma_start(out=wu_blk, in_=wu_d[ib])
                    if ib == KI - 3:
                        wd_blk = wd_pool.tile([P, KI, P], bf16, tag="wd",
                                              name=f"wd_{cls}_pre")
                        nc.sync.dma_start(out=wd_blk, in_=wd_d[0])
                        wd_pre.append(wd_blk)

                    ps_g = [ps_pool.tile([P, 512], f32, tag=f"pg{bi}",
                                         name=f"psg{bi}_{cls}_{ib}")
                            for bi in range(nblk)]
                    ps_u = [ps_pool.tile([P, 512], f32, tag=f"pu{bi}",
                                         name=f"psu{bi}_{cls}_{ib}")
                            for bi in range(nblk)] if not packed else []
                    for bi, (b0, bw) in enumerate(cblk):
                        for dk in range(KD):
                            nc.tensor.matmul(
                                ps_g[bi][:, :bw], lhsT=wg_blk[:, dk, :],
                                rhs=tok[dk][:, b0:b0 + bw],
                                start=dk == 0, stop=dk == KD - 1)
                    if not packed:
                        for bi, (b0, bw) in enumerate(cblk):
                            for dk in range(KD):
                                nc.tensor.matmul(
                                    ps_u[bi][:, :bw], lhsT=wu_blk[:, dk, :],
                                    rhs=tok[dk][:, b0:b0 + bw],
                                    start=dk == 0, stop=dk == KD - 1)
                    for bi, (b0, bw) in enumerate(cblk):
                        if packed:
                            tmp = tmp_pool.tile([IHALF, 512], f32,
                                                tag=f"t{bi}",
                                                name=f"tp{bi}_{cls}_{ib}")
                            nc.scalar.activation(tmp[:, :bw],
                                                 ps_g[bi][:IHALF, :bw], SILU)
                            tmp2 = tmp_pool.tile([IHALF, 512], f32,
                                                 tag=f"t2{bi}",
                                                 name=f"tq{bi}_{cls}_{ib}")
                            nc.vector.tensor_mul(tmp2[:, :bw], tmp[:, :bw],
                                                 ps_g[bi][IHALF:, :bw])
                            nc.vector.tensor_mul(
                                h[ib][:IHALF, b0:b0 + bw], tmp2[:, :bw],
                                wvr[:IHALF, b0:b0 + bw])
                        else:
                            tmp = tmp_pool.tile([P, 512], f32, tag=f"t{bi}",
                                                name=f"tp{bi}_{cls}_{ib}")
                            nc.scalar.activation(tmp[:, :bw],
                                                 ps_g[bi][:, :bw], SILU)
                            tmp2 = tmp_pool.tile([P, 512], f32, tag=f"t2{bi}",
                                                 name=f"tq{bi}_{cls}_{ib}")
                            nc.vector.tensor_mul(tmp2[:, :bw], tmp[:, :bw],
                                                 ps_u[bi][:, :bw])
                            nc.vector.tensor_mul(
                                h[ib][:, b0:b0 + bw], tmp2[:, :bw],
                                wvr[:, b0:b0 + bw])

                # foreign half-chunk: gate/up over cols [own_w:own_w+FOR_W)
                if has_foreign:
                    wfg_t = wgu_pool.tile([P, KD, P], bf16, tag="wg",
                                          name=f"wfg_{cls}")
                    nc.sync.dma_start(out=wfg_t, in_=wfg_d)
                    wfu_t = wgu_pool.tile([P, KD, P], bf16, tag="wu",
                                          name=f"wfu_{cls}")
                    nc.sync.dma_start(out=wfu_t, in_=wfu_d)
                    ps_fg = ps_pool.tile([P, 512], f32, tag="pg0",
                                         name=f"psfg_{cls}")
                    ps_fu = ps_pool.tile([P, 512], f32, tag="pu0",
                                         name=f"psfu_{cls}")
                    for dk in range(KD):
                        nc.tensor.matmul(
                            ps_fg[:, :FOR_W], lhsT=wfg_t[:, dk, :],
                            rhs=tok[dk][:, width:width + FOR_W],
                            start=dk == 0, stop=dk == KD - 1)
                    for dk in range(KD):
                        nc.tensor.matmul(
                            ps_fu[:, :FOR_W], lhsT=wfu_t[:, dk, :],
                            rhs=tok[dk][:, width:width + FOR_W],
                            start=dk == 0, stop=dk == KD - 1)
                    tmp = tmp_pool.tile([P, 512], f32, tag="t0",
                                        name=f"tpf_{cls}")
                    nc.scalar.activation(tmp[:, :FOR_W], ps_fg[:, :FOR_W],
                                         SILU)
                    tmp2 = tmp_pool.tile([P, 512], f32, tag="t20",
                                         name=f"tqf_{cls}")
                    nc.vector.tensor_mul(tmp2[:, :FOR_W], tmp[:, :FOR_W],
                                         ps_fu[:, :FOR_W])
                    nc.vector.tensor_mul(hF, tmp2[:, :FOR_W],
                                         wvr[:, width:width + FOR_W])

                # ---- phase 2: down matmul -> yT ----
                for dc in range(KO):
                    if dc == 0:
                        wd_blk = wd_pre[0]
                    else:
                        wd_blk = wd_pool.tile([P, KI, P], bf16, tag="wd",
                                              name=f"wd_{cls}_{dc}")
                        nc.sync.dma_start(out=wd_blk, in_=wd_d[dc])
                    ps_y = [ps_pool.tile([P, 512], f32, tag=f"pg{bi}",
                                         name=f"psy{bi}_{cls}_{dc}")
                            for bi in range(nblk)]
                    for bi, (b0, bw) in enumerate(cblk):
                        for ib in range(skip, KI):
                            nc.tensor.matmul(
                                ps_y[bi][:, :bw], lhsT=wd_blk[:, ib, :],
                                rhs=h[ib][:, b0:b0 + bw],
                                start=ib == skip, stop=ib == KI - 1)
                    if has_foreign:
                        ps_yf = ps_pool.tile([P, 512], f32, tag="pu0",
                                             name=f"psyf_{cls}_{dc}")
                        nc.tensor.matmul(ps_yf[:, :FOR_W],
                                         lhsT=wfd_sb[:, dc, :], rhs=hF,
                                         start=True, stop=True)
                    y_sb = y_pool.tile([P, ctmax], f32, tag="ysb",
                                       name=f"ysb_{cls}_{dc}")
                    for bi, (b0, bw) in enumerate(cblk):
                        nc.scalar.copy(y_sb[:, b0:b0 + bw], ps_y[bi][:, :bw])
                    if has_foreign:
                        nc.scalar.copy(y_sb[:, width:width + FOR_W],
                                       ps_yf[:, :FOR_W])
                    nc.sync.dma_start(out=yT_d[dc][:, :tok_w],
                                      in_=y_sb[:, :tok_w])

            # one If-region per body class; cores of the same class run
            # identical instructions on per-core data.  The light class is
            # emitted FIRST: sequencers walk untaken bodies at ~3.5ns/instr,
            # and a walk before a core's own body delays its start, while a
            # walk after overlaps the output drain -- measured cheaper.
            emit_order = sorted(specs, key=lambda s: (s["cls"] != "light",
                                                      s["width"]))
            for ci, sp in enumerate(emit_order):
                lo, hi = sp["pid_range"]
                cond = (pid < hi) if hi - lo > 1 else (pid == lo)
                with tc.If(cond):
                    emit_body(sp["cls"], sp["width"], sp["has_foreign"],
                              sp["skip"])

    nc.compile()
    return nc


def _prep_expert(w_gate_e, w_up_e, w_down_e):
    """Per-expert device weight layouts (packed last gate/up chunk)."""
    wg = np.zeros((DP, DP), np.float32)
    wg[:D, :D] = w_gate_e
    wu = np.zeros((DP, DP), np.float32)
    wu[:D, :D] = w_up_e
    # pack: last I chunk keeps gate[2816:2880] in cols 0:64 and gains
    # up[2816:2880] in cols 64:128
    wg[:, DP - P + IHALF:] = wu[:, DP - P:DP - P + IHALF]
    wgp = np.ascontiguousarray(
        wg.reshape(KD, P, KI, P).transpose(2, 1, 0, 3)).astype(BF16)
    wup = np.ascontiguousarray(
        wu.reshape(KD, P, KI, P).transpose(2, 1, 0, 3)[:KI - 1]).astype(BF16)

    wd = np.zeros((DP, DP), np.float32)
    wd[:D, :D] = w_down_e
    wdp = np.ascontiguousarray(
        wd.reshape(KI, P, KO, P).transpose(2, 1, 0, 3)).astype(BF16)
    return wgp, wup, wdp


def moe_forward(x, w_router, w_gate, w_up, w_down, trace=False):
    from concourse.bass_utils import run_bass_kernel_spmd

    x = np.asarray(x)
    t, i1, i2, w1, w2 = _route(x, np.asarray(w_router))
    Ttok = t.shape[0]

    idx_list, wv_list = [], []
    for e in range(E):
        sel1 = i1 == e
        sel2 = i2 == e
        idx = np.nonzero(sel1 | sel2)[0]
        w = np.where(sel1[idx], w1[idx], w2[idx]).astype(np.float32)
        idx_list.append(idx)
        wv_list.append(w)
    counts = [len(ix) for ix in idx_list]

    classes, order, zs, heavy, light = _schedule(counts)
    # pid layout: lights at pids [0, len(light)), then heavies
    # body specs: one light class + one class per heavy expert
    any_foreign = any(classes[e]["foreign"] is not None for e in light)
    hw_max = max((counts[e] for e in heavy), default=0)
    ctmax = max(LIGHT_W + (FOR_W if any_foreign else 0), hw_max)
    ctmax = (ctmax + 31) // 32 * 32

    specs = []
    if light:
        specs.append({"cls": "light", "width": LIGHT_W,
                      "has_foreign": any_foreign, "skip": 0,
                      "pid_range": (0, len(light))})
    for i, e in enumerate(heavy):
        specs.append({"cls": f"heavy{e}", "width": counts[e],
                      "has_foreign": False, "skip": classes[e]["donated"],
                      "pid_range": (len(light) + i, len(light) + i + 1)})
    h_w = max(s["width"] for s in specs)
    spec_key = tuple((s["cls"], s["width"], s["has_foreign"], s["skip"])
                     for s in specs)

    key = (spec_key, ctmax)
    if key not in _cache:
        _cache[key] = _build_program(spec_key, specs, ctmax, h_w)
    nc = _cache[key]

    wg_f = np.asarray(w_gate, np.float32)
    wu_f = np.asarray(w_up, np.float32)
    wd_f = np.asarray(w_down, np.float32)
    experts = {e: _prep_expert(wg_f[e], wu_f[e], wd_f[e]) for e in range(E)}

    zdum_g = np.zeros((P, KD, P), BF16)
    zdum_d = np.zeros((P, KO, P), BF16)

    in_maps = []
    core_info = []  # (own tok ids, n_own, foreign tok ids or None)
    for k in range(N_CORES):
        e = order[k]
        cl = classes[e]
        own_ids = idx_list[e]
        own_w = LIGHT_W if cl["kind"] == "light" else counts[e]
        wvals = wv_list[e]

        n = len(own_ids)
        tpad = np.zeros((ctmax, DP), np.float32)
        tpad[:n, :D] = t[own_ids]
        wv = np.zeros((ctmax,), np.float32)
        wv[:n] = wvals

        g1, u1, d1 = experts[e]
        fm = {"wg": g1, "wu": u1, "wd": d1,
              "wfg": zdum_g, "wfu": zdum_g, "wfd": zdum_d}
        f_ids = None
        if cl["kind"] == "light" and cl["foreign"] is not None:
            fe, fib, fb0, fbw = cl["foreign"]
            f_ids = idx_list[fe][fb0:fb0 + fbw]
            nf = len(f_ids)
            tpad[own_w:own_w + nf, :D] = t[f_ids]
            wv[own_w:own_w + nf] = wv_list[fe][fb0:fb0 + fbw]
            g2, u2, d2 = experts[fe]
            fm["wfg"] = np.ascontiguousarray(g2[fib])
            fm["wfu"] = np.ascontiguousarray(u2[fib])
            fm["wfd"] = np.ascontiguousarray(
                d2[:, :, fib, :].transpose(1, 0, 2))

        tT = np.ascontiguousarray(tpad.T).reshape(KD, P, ctmax).astype(BF16)
        wvr = np.ascontiguousarray(np.broadcast_to(wv, (P, ctmax)))
        fm.update({"tT": tT, "wvr": wvr})
        in_maps.append(fm)
        core_info.append((own_ids, n, own_w, f_ids))

    try:
        res = run_bass_kernel_spmd(nc, in_maps, list(range(N_CORES)),
                                   trace=trace)
    except Exception:
        # transient NRT/device hiccups have been observed; retry once
        res = run_bass_kernel_spmd(nc, in_maps, list(range(N_CORES)),
                                   trace=trace)

    out = np.zeros((Ttok, D), np.float32)
    for k in range(N_CORES):
        own_ids, n, own_w, f_ids = core_info[k]
        yT = res.results[k]["yT"].reshape(DP, -1)  # [dc*128+dp, c]
        np.add.at(out, own_ids, yT[:D, :n].T)
        if f_ids is not None and len(f_ids):
            np.add.at(out, f_ids, yT[:D, own_w:own_w + len(f_ids)].T)

    return out.reshape(x.shape).astype(np.float32), res


def kernel(x, w_router, w_gate, w_up, w_down):
    out, _ = moe_forward(x, w_router, w_gate, w_up, w_down,
                         trace=bool(int(os.environ.get("MOE_TRACE", "0"))))
    return out
